# revision 27
# baseline (speedup 1.0000x reference)
# Multi-head attention (B=2, S=2048, D=1024, H=16) on 8 TRN2 NeuronCores.
#
# Sharding: core c -> batch b = c//4, head group g = c%4 (4 heads = 256
# features). Each core computes its heads' attention for its batch plus the
# row-parallel slice of the output projection; the host sums the 4 partials
# per batch (the all-reduce) and adds bo.
#
# Structure (v2): the kernel is paced by ScalarE's exp chain (128 exps of
# [128,2,512] ~= 135 us), so everything else is scheduled around keeping
# that chain dense from ~12 us onward:
#   - DMAs are emitted in deadline order at (contraction-chunk, q-slice)
#     granularity, so the first logits tile only waits for ~3 MB.
#   - Only K qs0 + Q qs0 projections run in the lead; K qs1-3, the V
#     projection, and Q qs1-3 are interleaved into the attention k-chunk
#     loops at points where the PE has slack under the exp pace.
#   - The AV accumulation trails logits by PEND k-chunks and the last
#     CARRY chunks of each q-block are drained early in the next block,
#     which keeps 2 of the 4 PSUM slots free at block starts for the
#     output-projection and Q-projection interjections.
#   - Redundant (idempotent start/stop) logits matmuls are sprinkled into
#     PE-light stretches so HAM never re-throttles the PE clock to 1.2 GHz.
#
# Device math per core (layouts transposed so softmax needs no cross-
# partition reduce; all matmul operands fp16, accumulation fp32 in PSUM):
#   qhT[f, s] = wq_g @ q_b^T ; khT, vhT likewise     (fp16 matmuls)
#   vh slots per head: [ones 64 | v 64]              (denominator trick)
#   logitsT[k, q] = khT_h-slices^T @ qhT_h           (K=64, pairs row-packed,
#                                                     T0/T8 tile concurrency)
#   expT = exp(logitsT / 8) * (1 - mask)^T           (ACT exp + DVE mask mult)
#   av[:, q] = vh_slot^T @ expT                      (rows 0:64 = denominator
#                                                     replicated, 64:128 = out)
#   attnN2 = av_out * recip(av_denom)                (both heads packed in one
#                                                     128-part tile; the two
#                                                     cross-partition moves per
#                                                     pair run on ScalarE)
#   partial[q, D] = attnN2^T @ wo2[pair]             (K=128 packed pairs)
import os
import numpy as np

B, S, DM, H, DEPTH = 2, 2048, 1024, 16, 64
NCORES = 8
GROUPS = 4            # head-groups per batch == cores per batch
HG = H // GROUPS      # heads per core
FS = HG * DEPTH       # features per core
QC = 512              # q-block (matmul free dim)
NQC = S // QC
NKC = S // 128        # k chunks
PAIRS = HG // 2
CCH = DM // 128       # contraction chunks for the projections

_CACHE = {}

# Tunables (PE filler / pipeline depths)
PEND = int(os.environ.get("PEND", "10"))     # AV lag in k-chunks
CARRY = 16 - ((NKC - PEND) + 2 * 4 - 4)      # = 6 with PEND=10 (see pops below)
DUP_B = int(os.environ.get("DUP_B", "6"))    # dup pairs, AVC-only kcs (blocks 2-3)
DUP_NORM = int(os.environ.get("DUP_NORM", "5"))  # dup pairs at the NORM kc
DUP_C = int(os.environ.get("DUP_C", "2"))    # dup pairs, late kcs blocks 1-3
DUP_N = int(os.environ.get("DUP_N", "512"))  # dup matmul free dim
DUP_B0 = int(os.environ.get("DUP_B0", "2"))  # dup pairs per (kc,pair), block 0
WARM_K = int(os.environ.get("WARM_K", "4"))  # warmup dummies per K qs0 chunk
WARM_Q = int(os.environ.get("WARM_Q", "2"))  # warmup dummies per Q qs0 chunk
TAIL = int(os.environ.get("TAIL", "10"))     # tail dup pairs (keep WO warm)


def _build():
    import concourse.tile as tile
    from concourse import bacc, mybir

    dt = mybir.dt
    f32, f16 = dt.float32, dt.float16
    Act = mybir.ActivationFunctionType

    nc = bacc.Bacc("TRN2", target_bir_lowering=False, debug=False,
                   num_devices=NCORES)

    xq = nc.dram_tensor("xq", [DM, S], f16, kind="ExternalInput").ap()
    xk = nc.dram_tensor("xk", [DM, S], f16, kind="ExternalInput").ap()
    xv = nc.dram_tensor("xv", [DM, S], f16, kind="ExternalInput").ap()
    wqd = nc.dram_tensor("wq", [128, CCH, FS], f16, kind="ExternalInput").ap()
    wkd = nc.dram_tensor("wk", [128, CCH, FS], f16, kind="ExternalInput").ap()
    wvd = nc.dram_tensor("wv", [128, CCH, FS], f16, kind="ExternalInput").ap()
    wod = nc.dram_tensor("wo", [PAIRS, 128, DM], f16, kind="ExternalInput").ap()
    m01 = nc.dram_tensor("m01", [S, S], f16, kind="ExternalInput").ap()
    bqd = nc.dram_tensor("bq", [128, 2], f32, kind="ExternalInput").ap()
    bkd = nc.dram_tensor("bk", [128, 2], f32, kind="ExternalInput").ap()
    out = nc.dram_tensor("part", [S, DM], f16, kind="ExternalOutput").ap()

    with tile.TileContext(nc) as tc:
        with (
            tc.tile_pool(name="xp", bufs=24) as xp,
            tc.tile_pool(name="wp", bufs=2) as wp,
            tc.tile_pool(name="wvp", bufs=1) as wvp,
            tc.tile_pool(name="wop", bufs=2) as wop,
            tc.tile_pool(name="qk", bufs=4) as qkp,
            tc.tile_pool(name="vh", bufs=16) as vp,
            tc.tile_pool(name="mk", bufs=18) as mkp,
            tc.tile_pool(name="ex", bufs=4) as exp_p,
            tc.tile_pool(name="exm", bufs=34) as exm_p,
            tc.tile_pool(name="au", bufs=4) as aup,
            tc.tile_pool(name="rc", bufs=4) as rcp,
            tc.tile_pool(name="an", bufs=4) as anp,
            tc.tile_pool(name="os", bufs=4) as osp,
            tc.tile_pool(name="cst", bufs=4) as cst,
            tc.tile_pool(name="ps", bufs=4, space="PSUM") as psp,
        ):
            def big():
                return psp.tile([128, 2, QC], f32, tag="big", name="big")

            # ---------- input DMA emitters (deadline-ordered below) ----
            xt = {}   # (tensor_name, c, qs) -> [128, QC] f16 tile

            def x_dma(src, key, c, qs):
                t = xp.tile([128, QC], f16, tag="x", name="x")
                nc.sync.dma_start(
                    t[:], src[128 * c:128 * (c + 1), QC * qs:QC * (qs + 1)])
                xt[(key, c, qs)] = t

            mk = {}   # (qcb, kc) -> [128, QC] f16 mask tile

            def mk_dma(qcb, kc):
                t = mkp.tile([128, QC], f16, tag="mk", name="mk")
                nc.sync.dma_start(
                    t[:], m01[128 * kc:128 * (kc + 1),
                              QC * qcb:QC * (qcb + 1)])
                mk[(qcb, kc)] = t

            # ---------- lead: only what gates the first exp ----------
            wk_t = wp.tile([128, CCH, FS], f16, tag="w", name="w")
            nc.sync.dma_start(wk_t[:], wkd[:])
            bk_t = cst.tile([128, 2], f32, tag="bias", name="bias")
            nc.sync.dma_start(bk_t[:], bkd[:])
            for c in range(CCH):
                x_dma(xk, "k", c, 0)
            wq_t = wp.tile([128, CCH, FS], f16, tag="w", name="w")
            nc.sync.dma_start(wq_t[:], wqd[:])
            bq_t = cst.tile([128, 2], f32, tag="bias", name="bias")
            nc.sync.dma_start(bq_t[:], bqd[:])
            for c in range(CCH):
                x_dma(xq, "q", c, 0)
            for kc in range(4):
                mk_dma(0, kc)

            # vh tiles pre-allocated; ones slots memset on the idle DVE
            vh = [vp.tile([128, HG, 128], f16, tag="vh", name="vh")
                  for _ in range(NKC)]
            for kr in range(NKC):
                nc.vector.memset(vh[kr][:, :, 0:64], 1.0)

            qhT = [qkp.tile([128, S], f16, tag="qk", name="qk")
                   for _ in range(2)]
            khT = [qkp.tile([128, S], f16, tag="qk", name="qk")
                   for _ in range(2)]

            # ---------- projection emitters ----------
            # warmup dummies: garbage matmuls on the already-resident weight
            # tile, written to a dead PSUM slot.  They keep the PE array busy
            # through the lead's DMA waits so HAM un-throttles to 2.4 GHz
            # before the real projections run.
            wu = psp.tile([128, 2, QC], f32, tag="big", name="big")

            def warm_mm(n):
                for i in range(n):
                    nc.tensor.matmul(
                        wu[:, i % 2, 0:FS],
                        lhsT=wk_t[:, i % CCH, 0:128],
                        rhs=wk_t[:, (i + 1) % CCH, :],
                        start=True, stop=True, skip_group_check=True)

            def proj_piece(dst, w_t, key, qs, m, bias_t, on_act, warm=0):
                # one m-half (128 partitions of K (pair m), one q-slice)
                qsl = slice(QC * qs, QC * (qs + 1))
                p = big()
                for c in range(CCH):
                    nc.tensor.matmul(
                        p[:, 0, :],
                        lhsT=w_t[:, c, 128 * m:128 * (m + 1)],
                        rhs=xt[(key, c, qs)][:],
                        start=(c == 0), stop=(c == CCH - 1),
                        skip_group_check=warm > 0)
                    if warm:
                        warm_mm(warm)
                if on_act:
                    nc.scalar.add(dst[m][:, qsl], p[:, 0, :],
                                  bias_t[:, m:m + 1])
                else:
                    nc.vector.tensor_scalar_add(dst[m][:, qsl], p[:, 0, :],
                                                bias_t[:, m:m + 1])

            def v_kr(kr):
                qs, r = divmod(kr, 4)
                pv = psp.tile([128, 2, QC], f32, tag="big", name="big")
                for c in range(CCH):
                    nc.tensor.matmul(
                        pv[:, 0, 0:FS],
                        lhsT=xt[("v", c, qs)][:, 128 * r:128 * (r + 1)],
                        rhs=wv_t[:, c, :],
                        start=(c == 0), stop=(c == CCH - 1))
                nc.vector.tensor_copy(
                    vh[kr][:, :, 64:128],
                    pv[:, 0, 0:FS].rearrange("p (h d) -> p h d", d=DEPTH))

            # ---------- attention unit (logits + dups + exp + mask) ----
            pend_all = {0: {p: [] for p in range(PAIRS)}}

            def attn_unit(qcb, kc, pair, dups):
                qsl = slice(QC * qcb, QC * (qcb + 1))
                ksl = slice(128 * kc, 128 * (kc + 1))
                lg2 = big()
                for half in range(2):
                    psl = slice(64 * half, 64 * (half + 1))
                    nc.tensor.matmul(
                        lg2[:, half, :],
                        lhsT=khT[pair][psl, ksl],
                        rhs=qhT[pair][psl, qsl],
                        start=True, stop=True)
                # idempotent dup matmuls: PE-warmth filler in stretches
                # where the PE would otherwise idle and HAM would throttle
                for _ in range(dups):
                    for half in range(2):
                        psl = slice(64 * half, 64 * (half + 1))
                        nc.tensor.matmul(
                            lg2[:, half, 0:DUP_N],
                            lhsT=khT[pair][psl, ksl],
                            rhs=qhT[pair][psl, QC * qcb:QC * qcb + DUP_N],
                            start=True, stop=True)
                ex2 = exp_p.tile([128, 2, QC], f16, tag="ex", name="ex")
                nc.scalar.activation(ex2[:], lg2[:], Act.Exp, scale=0.125)
                exm2 = exm_p.tile([128, 2, QC], f16, tag="exm", name="exm")
                mbc = (mk[(qcb, kc)][:]
                       .rearrange("p (o q) -> p o q", o=1)
                       .to_broadcast((128, 2, QC)))
                nc.vector.tensor_mul(exm2[:], ex2[:], mbc)
                pend_all[qcb][pair].append((qcb, pair, kc, exm2))

            # ---------- lead projections (ACT idle here, bias on ACT).
            # m0 halves + the first attention unit first, so the exp chain
            # starts as soon as the m0 pieces and first mask are in.
            proj_piece(khT, wk_t, "k", 0, 0, bk_t, True, warm=WARM_K)
            proj_piece(qhT, wq_t, "q", 0, 0, bq_t, True, warm=WARM_Q)
            attn_unit(0, 0, 0, 0)
            proj_piece(khT, wk_t, "k", 0, 1, bk_t, True)
            proj_piece(qhT, wq_t, "q", 0, 1, bq_t, True)

            # ---------- rest of the input DMAs, deadline order ----------
            for c in range(CCH):
                x_dma(xk, "k", c, 1)
            wv_t = wvp.tile([128, CCH, FS], f16, tag="w", name="w")
            nc.sync.dma_start(wv_t[:], wvd[:])
            for c in range(CCH):
                x_dma(xv, "v", c, 0)
            for kc in range(4, 8):
                mk_dma(0, kc)
            for c in range(CCH):
                x_dma(xk, "k", c, 2)
            for c in range(CCH):
                x_dma(xv, "v", c, 1)
            for kc in range(8, 12):
                mk_dma(0, kc)
            for c in range(CCH):
                x_dma(xk, "k", c, 3)
            for c in range(CCH):
                x_dma(xv, "v", c, 2)
            wo_t = []
            for p in range(PAIRS):
                t = wop.tile([128, DM], f16, tag="wo", name="wo")
                nc.sync.dma_start(t[:], wod[p])
                wo_t.append(t)
            for c in range(CCH):
                x_dma(xq, "q", c, 1)
            for c in range(CCH):
                x_dma(xv, "v", c, 3)
            for kc in range(12, 16):
                mk_dma(0, kc)

            # ---------- attention ----------
            av2 = {}      # qcb -> [pair tiles]
            attnN2 = {}   # qcb -> [pair an tiles]

            def emit_av(qcb, pair, dk, exm2):
                if qcb not in av2:   # lazy: bind PSUM slots at first use
                    av2[qcb] = [big() for _ in range(PAIRS)]
                for half in range(2):
                    nc.tensor.matmul(
                        av2[qcb][pair][:, half, :],
                        lhsT=vh[dk][:, 2 * pair + half, :],
                        rhs=exm2[:, half, :],
                        start=(dk == 0), stop=(dk == NKC - 1),
                        skip_group_check=True)

            def normalize(qcb, pair, act_ok=False):
                # ScalarE is the bottleneck engine mid-kernel, so the two
                # 64-partition shifts go DVE -> (SBUF-to-SBUF DMA) instead
                # of scalar.copy; in the tail ACT is idle and helps out.
                av = av2[qcb][pair]
                au2 = aup.tile([128, QC], f32, tag="au", name="au")
                rc2 = rcp.tile([128, QC], f32, tag="rc", name="rc")
                if act_ok:
                    nc.scalar.copy(au2[0:64, :], av[64:128, 0, :])
                    nc.scalar.copy(au2[64:128, :], av[64:128, 1, :])
                else:
                    tmp = aup.tile([128, QC], f32, tag="au", name="au")
                    nc.vector.tensor_copy(tmp[64:128, :], av[64:128, 0, :])
                    nc.sync.dma_start(au2[0:64, :], tmp[64:128, :])
                    nc.vector.tensor_copy(au2[64:128, :], av[64:128, 1, :])
                nc.vector.reciprocal_approx_fast(rc2[0:64, :], av[0:64, 0, :])
                rcb = rcp.tile([128, QC], f32, tag="rc", name="rc")
                nc.vector.reciprocal_approx_fast(rcb[0:64, :], av[0:64, 1, :])
                nc.sync.dma_start(rc2[64:128, :], rcb[0:64, :])
                an2 = anp.tile([128, QC], f16, tag="an", name="an")
                nc.vector.tensor_mul(an2[:], au2[:], rc2[:])
                return an2

            def emit_wo_qm(qcb, qm, act_cast=False):
                row = slice(128 * (4 * qcb + qm), 128 * (4 * qcb + qm + 1))
                po = big()
                for dn in range(2):
                    dsl = slice(512 * dn, 512 * (dn + 1))
                    for p in range(PAIRS):
                        nc.tensor.matmul(
                            po[:, dn, :],
                            lhsT=attnN2[qcb][p][:, 128 * qm:128 * (qm + 1)],
                            rhs=wo_t[p][:, dsl],
                            start=(p == 0), stop=(p == PAIRS - 1))
                ot = osp.tile([128, 2, 512], f16, tag="os", name="os")
                if act_cast:
                    nc.scalar.copy(ot[:], po[:])
                else:
                    nc.vector.tensor_copy(ot[:], po[:])
                nc.sync.dma_start(
                    out[row, :].rearrange("p (o q) -> p o q", o=2), ot[:])

            # Hook tables: extra PE work interleaved into the kc loops.
            # ("K",qs,m) / ("Q",qs,m): projection halves; ("V",kr): V chunk;
            # ("AVC",): drain up to 2 carried AVs per pair; ("NORM",qcb):
            # both normalizes; ("WO",qcb,qm): output projection block;
            # ("XDMA",key,qs): deferred input DMA emission.
            #
            # Block-lagged AV: block q's AVs all run in block q+1 (drained
            # 2/pair at kc0-7).  That evens PE load across blocks (block 0
            # is full of K/Q/V projections) and means only ONE av2
            # generation is PSUM-live at a time: lg rotation (2 slots) +
            # av2[q-1] (2 slots), with WO/Q-proj tiles reusing av2[q-1]'s
            # slots right after NORM frees them at kc8.
            hooks = {}
            # block 0: hooks run one kc AFTER emission of that kc's
            # logits, so a DMA-late projection piece never heads-of-line
            # blocks the next logits (the lg rotation can run ~2 kc deep
            # while av2 is unbound).  Deadlines: K qs_s by kc 4s, V kr by
            # its AVC pop in block 1, Q1 by b1 kc0.
            hooks[(0, 2)] = [("K", 1, 0)]
            hooks[(0, 3)] = [("K", 1, 1), ("V", 0)]
            hooks[(0, 4)] = [("K", 2, 0), ("V", 1)]
            hooks[(0, 5)] = [("K", 2, 1), ("V", 2)]
            hooks[(0, 6)] = [("K", 3, 0), ("V", 3)]
            hooks[(0, 7)] = [("K", 3, 1), ("V", 4)]
            hooks[(0, 8)] = [("Q", 1, 0), ("V", 5)]
            hooks[(0, 9)] = [("Q", 1, 1), ("V", 6)]
            hooks[(0, 10)] = [("V", 7), ("V", 8), ("XDMA", "q", 2)]
            hooks[(0, 11)] = [("V", 9), ("V", 10)]
            hooks[(0, 12)] = [("V", 11), ("V", 12)]
            hooks[(0, 13)] = [("V", 13)]
            # blocks 1-3: drain prev block's AVs with slack (x2 only where
            # nothing else runs), normalize at kc9, WO at kc10-13 (reusing
            # freed av2 slots), next-block Q projection at kc14/15
            avc_rate = {}
            for q in (1, 2, 3):
                rates = [2, 2, 2, 2, 2, 2, 2, 1, 1] if q > 1 else                         [1, 1, 2, 2, 2, 2, 2, 2, 2]
                for kc, r in enumerate(rates):
                    hooks.setdefault((q, kc), []).append(("AVC", r))
                hooks.setdefault((q, 9), []).append(("NORM", q - 1))
                for i, kc in enumerate((10, 11, 12, 13)):
                    hooks.setdefault((q, kc), []).append(("WO", q - 1, i))
            hooks.setdefault((1, 0), []).insert(0, ("V", 14))
            hooks.setdefault((1, 1), []).insert(0, ("V", 15))
            hooks.setdefault((1, 2), []).append(("XDMA", "q", 3))
            hooks.setdefault((1, 14), []).append(("Q", 2, 0))
            hooks.setdefault((1, 15), []).append(("Q", 2, 1))
            hooks.setdefault((2, 14), []).append(("Q", 3, 0))
            hooks.setdefault((2, 15), []).append(("Q", 3, 1))

            carry = {p: [] for p in range(PAIRS)}   # AVs deferred across blocks

            def run_hook(h):
                if h[0] == "K":
                    proj_piece(khT, wk_t, "k", h[1], h[2], bk_t, False)
                elif h[0] == "Q":
                    proj_piece(qhT, wq_t, "q", h[1], h[2], bq_t, False)
                elif h[0] == "V":
                    v_kr(h[1])
                elif h[0] == "AVC":
                    for p in range(PAIRS):
                        for _ in range(h[1]):
                            if carry[p]:
                                emit_av(*carry[p].pop(0))
                elif h[0] == "NORM":
                    attnN2[h[1]] = [normalize(h[1], p) for p in range(PAIRS)]
                elif h[0] == "WO":
                    emit_wo_qm(h[1], h[2])
                elif h[0] == "XDMA":
                    for c in range(CCH):
                        x_dma(xq, h[1], c, h[2])

            def dup_count(qcb, kc):
                hs = [h[0] for h in hooks.get((qcb, kc), [])]
                if qcb == 0:
                    return DUP_B0
                if qcb == NQC - 1 and kc >= 9:
                    return 0    # b3 kc9+ is filled by its own AV pops
                if "NORM" in hs:
                    return DUP_NORM
                if any(x in hs for x in ("Q", "K", "V")):
                    return 0
                if "WO" in hs:
                    return DUP_C
                if ("AVC", 1) in hooks.get((qcb, kc), []):
                    return 2    # half-rate AVC kcs have ~800ns slack
                if kc < 9:      # full-rate AVC region
                    return 0
                return DUP_B    # bare kcs

            # own-block AV pops: only block 3 drains itself, so the tail
            # stays short; blocks 0-2 carry everything forward
            def own_pops(qcb, kc):
                if qcb != NQC - 1:
                    return 0
                if kc == 9:
                    return 2     # NORM kc: no other PE work
                if 10 <= kc <= 12:
                    return 1
                if kc == 13:
                    return 2
                if kc >= 14:
                    return 3     # ACT is done; PE-over is free here
                return 0

            for qcb in range(NQC):
                pend = pend_all.setdefault(
                    qcb, {p: [] for p in range(PAIRS)})
                last = qcb == NQC - 1
                for kc in range(NKC):
                    if qcb < NQC - 1 and kc >= 8:  # prefetch next block masks
                        mk_dma(qcb + 1, 2 * (kc - 8))
                        mk_dma(qcb + 1, 2 * (kc - 8) + 1)
                    pops = own_pops(qcb, kc)
                    dups = dup_count(qcb, kc)
                    for pair in range(PAIRS):
                        if (qcb, kc, pair) != (0, 0, 0):  # pre-emitted in lead
                            attn_unit(qcb, kc, pair, dups)
                        for _ in range(pops):
                            if pend[pair]:
                                emit_av(*pend[pair].pop(0))
                    for h in hooks.get((qcb, kc), []):
                        run_hook(h)
                for p in range(PAIRS):
                    carry[p] = pend[p]

            # tail: drain the rest of block 3's AVs with PE fill through
            # the final normalize, then the last WO blocks
            for p in range(PAIRS):
                while pend_all[NQC - 1][p]:
                    emit_av(*pend_all[NQC - 1][p].pop(0))
            wu2 = big()
            for i in range(TAIL):
                nc.tensor.matmul(
                    wu2[:, i % 2, :], lhsT=khT[0][:, 0:128],
                    rhs=khT[0][:, 0:QC],
                    start=True, stop=True, skip_group_check=True)
            attnN2[NQC - 1] = [normalize(NQC - 1, p, act_ok=True)
                               for p in range(PAIRS)]
            for qm in range(4):
                emit_wo_qm(NQC - 1, qm, act_cast=qm % 2 == 1)

    nc.compile()
    return nc


def _get_program():
    if "nc" not in _CACHE:
        _CACHE["nc"] = _build()
    return _CACHE["nc"]


def _in_maps(q, k, v, mask, wq, bq, wk, bk, wv, bv, wo, bo):
    q = np.asarray(q, np.float32)
    k = np.asarray(k, np.float32)
    v = np.asarray(v, np.float32)
    mask = np.asarray(mask, np.float32)
    wq = np.asarray(wq, np.float32)
    wk = np.asarray(wk, np.float32)
    wv = np.asarray(wv, np.float32)
    wo = np.asarray(wo, np.float32)
    bq = np.asarray(bq, np.float32)
    bk = np.asarray(bk, np.float32)
    bv = np.asarray(bv, np.float32)
    assert np.all(bv == 0.0), "nonzero bv not supported by this kernel"

    def wdev(w, cols):
        # [128, CCH, FS] layout: partition p, contraction chunk c holds
        # dram row 128*c + p of w[cols].T
        wT = np.ascontiguousarray(w[cols].T).astype(np.float16)
        return np.ascontiguousarray(
            wT.reshape(CCH, 128, FS).transpose(1, 0, 2))

    maps = []
    xqT = [np.ascontiguousarray(q[b].T).astype(np.float16) for b in range(B)]
    xkT = [np.ascontiguousarray(k[b].T).astype(np.float16) for b in range(B)]
    xvT = [np.ascontiguousarray(v[b].T).astype(np.float16) for b in range(B)]
    m01 = [np.ascontiguousarray((1.0 - mask[b, 0]).T).astype(np.float16)
           for b in range(B)]
    for c in range(NCORES):
        b, g = divmod(c, GROUPS)
        cols = slice(FS * g, FS * (g + 1))
        maps.append({
            "xq": xqT[b], "xk": xkT[b], "xv": xvT[b],
            "wq": wdev(wq, cols),
            "wk": wdev(wk, cols),
            "wv": wdev(wv, cols),
            "wo": np.ascontiguousarray(
                wo[:, cols].T.reshape(PAIRS, 128, DM)).astype(np.float16),
            "m01": m01[b],
            "bq": np.ascontiguousarray(bq[cols].reshape(2, 128).T),
            "bk": np.ascontiguousarray(bk[cols].reshape(2, 128).T),
        })
    return maps


def _run(maps, trace=False):
    from concourse.bass_utils import run_bass_kernel_spmd
    nc = _get_program()
    kwargs = {}
    if trace:
        kwargs = dict(trace=True, tmpdir=os.environ.get("KERNEL_TRACE_DIR"))
    return run_bass_kernel_spmd(nc, maps, list(range(NCORES)), **kwargs)


def kernel(q, k, v, mask, wq, bq, wk, bk, wv, bv, wo, bo):
    maps = _in_maps(q, k, v, mask, wq, bq, wk, bk, wv, bv, wo, bo)
    res = _run(maps)
    parts = [res.results[c]["part"].astype(np.float32) for c in range(NCORES)]
    bo = np.asarray(bo, np.float32)
    outb = [parts[GROUPS * b] + parts[GROUPS * b + 1]
            + parts[GROUPS * b + 2] + parts[GROUPS * b + 3] + bo
            for b in range(B)]
    return np.stack(outb, 0).astype(np.float32)


# revision 28
# speedup vs baseline: 1.0300x; 1.0300x over previous
# Multi-head attention (B=2, S=2048, D=1024, H=16) on 8 TRN2 NeuronCores.
#
# Sharding: core c -> batch b = c//4, head group g = c%4 (4 heads = 256
# features). Each core computes its heads' attention for its batch plus the
# row-parallel slice of the output projection; the host sums the 4 partials
# per batch (the all-reduce) and adds bo.
#
# Structure (v2): the kernel is paced by ScalarE's exp chain (128 exps of
# [128,2,512] ~= 135 us), so everything else is scheduled around keeping
# that chain dense from ~12 us onward:
#   - DMAs are emitted in deadline order at (contraction-chunk, q-slice)
#     granularity, so the first logits tile only waits for ~3 MB.
#   - Only K qs0 + Q qs0 projections run in the lead; K qs1-3, the V
#     projection, and Q qs1-3 are interleaved into the attention k-chunk
#     loops at points where the PE has slack under the exp pace.
#   - The AV accumulation trails logits by PEND k-chunks and the last
#     CARRY chunks of each q-block are drained early in the next block,
#     which keeps 2 of the 4 PSUM slots free at block starts for the
#     output-projection and Q-projection interjections.
#   - Redundant (idempotent start/stop) logits matmuls are sprinkled into
#     PE-light stretches so HAM never re-throttles the PE clock to 1.2 GHz.
#
# Device math per core (layouts transposed so softmax needs no cross-
# partition reduce; all matmul operands fp16, accumulation fp32 in PSUM):
#   qhT[f, s] = wq_g @ q_b^T ; khT, vhT likewise     (fp16 matmuls)
#   vh slots per head: [ones 64 | v 64]              (denominator trick)
#   logitsT[k, q] = khT_h-slices^T @ qhT_h           (K=64, pairs row-packed,
#                                                     T0/T8 tile concurrency)
#   expT = exp(logitsT / 8) * (1 - mask)^T           (ACT exp + DVE mask mult)
#   av[:, q] = vh_slot^T @ expT                      (rows 0:64 = denominator
#                                                     replicated, 64:128 = out)
#   attnN2 = av_out * recip(av_denom)                (both heads packed in one
#                                                     128-part tile; the two
#                                                     cross-partition moves per
#                                                     pair run on ScalarE)
#   partial[q, D] = attnN2^T @ wo2[pair]             (K=128 packed pairs)
import os
import numpy as np

B, S, DM, H, DEPTH = 2, 2048, 1024, 16, 64
NCORES = 8
GROUPS = 4            # head-groups per batch == cores per batch
HG = H // GROUPS      # heads per core
FS = HG * DEPTH       # features per core
QC = 512              # q-block (matmul free dim)
NQC = S // QC
NKC = S // 128        # k chunks
PAIRS = HG // 2
CCH = DM // 128       # contraction chunks for the projections

_CACHE = {}

# Tunables (PE filler / pipeline depths)
PEND = int(os.environ.get("PEND", "10"))     # AV lag in k-chunks
CARRY = 16 - ((NKC - PEND) + 2 * 4 - 4)      # = 6 with PEND=10 (see pops below)
DUPSCALE = int(os.environ.get("DUPSCALE", "100"))  # % of PE-slack to fill
DUP_N = int(os.environ.get("DUP_N", "512"))  # dup matmul free dim
DUP_B0 = int(os.environ.get("DUP_B0", "2"))  # dup pairs per (kc,pair), block 0
WARM_K = int(os.environ.get("WARM_K", "4"))  # warmup dummies per K qs0 chunk
WARM_Q = int(os.environ.get("WARM_Q", "2"))  # warmup dummies per Q qs0 chunk
TAIL = int(os.environ.get("TAIL", "10"))     # tail dup pairs (keep WO warm)


def _build():
    import concourse.tile as tile
    from concourse import bacc, mybir

    dt = mybir.dt
    f32, f16 = dt.float32, dt.float16
    Act = mybir.ActivationFunctionType

    nc = bacc.Bacc("TRN2", target_bir_lowering=False, debug=False,
                   num_devices=NCORES)

    xq = nc.dram_tensor("xq", [DM, S], f16, kind="ExternalInput").ap()
    xk = nc.dram_tensor("xk", [DM, S], f16, kind="ExternalInput").ap()
    xv = nc.dram_tensor("xv", [DM, S], f16, kind="ExternalInput").ap()
    wqd = nc.dram_tensor("wq", [128, CCH, FS], f16, kind="ExternalInput").ap()
    wkd = nc.dram_tensor("wk", [128, CCH, FS], f16, kind="ExternalInput").ap()
    wvd = nc.dram_tensor("wv", [128, CCH, FS], f16, kind="ExternalInput").ap()
    wod = nc.dram_tensor("wo", [PAIRS, 128, DM], f16, kind="ExternalInput").ap()
    m01 = nc.dram_tensor("m01", [S, S], f16, kind="ExternalInput").ap()
    bqd = nc.dram_tensor("bq", [128, 2], f32, kind="ExternalInput").ap()
    bkd = nc.dram_tensor("bk", [128, 2], f32, kind="ExternalInput").ap()
    out = nc.dram_tensor("part", [S, DM], f16, kind="ExternalOutput").ap()

    with tile.TileContext(nc) as tc:
        with (
            tc.tile_pool(name="xp", bufs=24) as xp,
            tc.tile_pool(name="wp", bufs=2) as wp,
            tc.tile_pool(name="wvp", bufs=1) as wvp,
            tc.tile_pool(name="wop", bufs=2) as wop,
            tc.tile_pool(name="qk", bufs=4) as qkp,
            tc.tile_pool(name="vh", bufs=16) as vp,
            tc.tile_pool(name="mk", bufs=18) as mkp,
            tc.tile_pool(name="ex", bufs=4) as exp_p,
            tc.tile_pool(name="exm", bufs=34) as exm_p,
            tc.tile_pool(name="au", bufs=4) as aup,
            tc.tile_pool(name="rc", bufs=4) as rcp,
            tc.tile_pool(name="an", bufs=4) as anp,
            tc.tile_pool(name="os", bufs=4) as osp,
            tc.tile_pool(name="cst", bufs=4) as cst,
            tc.tile_pool(name="ps", bufs=4, space="PSUM") as psp,
        ):
            def big():
                return psp.tile([128, 2, QC], f32, tag="big", name="big")

            # ---------- input DMA emitters (deadline-ordered below) ----
            xt = {}   # (tensor_name, c, qs) -> [128, QC] f16 tile

            def x_dma(src, key, c, qs):
                t = xp.tile([128, QC], f16, tag="x", name="x")
                nc.sync.dma_start(
                    t[:], src[128 * c:128 * (c + 1), QC * qs:QC * (qs + 1)])
                xt[(key, c, qs)] = t

            mk = {}   # (qcb, kc) -> [128, QC] f16 mask tile

            def mk_dma(qcb, kc):
                t = mkp.tile([128, QC], f16, tag="mk", name="mk")
                nc.sync.dma_start(
                    t[:], m01[128 * kc:128 * (kc + 1),
                              QC * qcb:QC * (qcb + 1)])
                mk[(qcb, kc)] = t

            # ---------- lead: only what gates the first exp ----------
            wk_t = wp.tile([128, CCH, FS], f16, tag="w", name="w")
            nc.sync.dma_start(wk_t[:], wkd[:])
            bk_t = cst.tile([128, 2], f32, tag="bias", name="bias")
            nc.sync.dma_start(bk_t[:], bkd[:])
            for c in range(CCH):
                x_dma(xk, "k", c, 0)
            wq_t = wp.tile([128, CCH, FS], f16, tag="w", name="w")
            nc.sync.dma_start(wq_t[:], wqd[:])
            bq_t = cst.tile([128, 2], f32, tag="bias", name="bias")
            nc.sync.dma_start(bq_t[:], bqd[:])
            for c in range(CCH):
                x_dma(xq, "q", c, 0)
            for kc in range(4):
                mk_dma(0, kc)

            # vh tiles pre-allocated; ones slots memset on the idle DVE
            vh = [vp.tile([128, HG, 128], f16, tag="vh", name="vh")
                  for _ in range(NKC)]
            for kr in range(NKC):
                nc.vector.memset(vh[kr][:, :, 0:64], 1.0)

            qhT = [qkp.tile([128, S], f16, tag="qk", name="qk")
                   for _ in range(2)]
            khT = [qkp.tile([128, S], f16, tag="qk", name="qk")
                   for _ in range(2)]

            # ---------- projection emitters ----------
            # warmup dummies: garbage matmuls on the already-resident weight
            # tile, written to a dead PSUM slot.  They keep the PE array busy
            # through the lead's DMA waits so HAM un-throttles to 2.4 GHz
            # before the real projections run.
            wu = psp.tile([128, 2, QC], f32, tag="big", name="big")

            def warm_mm(n):
                for i in range(n):
                    nc.tensor.matmul(
                        wu[:, i % 2, 0:FS],
                        lhsT=wk_t[:, i % CCH, 0:128],
                        rhs=wk_t[:, (i + 1) % CCH, :],
                        start=True, stop=True, skip_group_check=True)

            def proj_piece(dst, w_t, key, qs, m, bias_t, on_act, warm=0):
                # one m-half (128 partitions of K (pair m), one q-slice)
                qsl = slice(QC * qs, QC * (qs + 1))
                p = big()
                for c in range(CCH):
                    nc.tensor.matmul(
                        p[:, 0, :],
                        lhsT=w_t[:, c, 128 * m:128 * (m + 1)],
                        rhs=xt[(key, c, qs)][:],
                        start=(c == 0), stop=(c == CCH - 1),
                        skip_group_check=warm > 0)
                    if warm:
                        warm_mm(warm)
                if on_act:
                    nc.scalar.add(dst[m][:, qsl], p[:, 0, :],
                                  bias_t[:, m:m + 1])
                else:
                    nc.vector.tensor_scalar_add(dst[m][:, qsl], p[:, 0, :],
                                                bias_t[:, m:m + 1])

            def v_kr(kr):
                qs, r = divmod(kr, 4)
                pv = psp.tile([128, 2, QC], f32, tag="big", name="big")
                for c in range(CCH):
                    nc.tensor.matmul(
                        pv[:, 0, 0:FS],
                        lhsT=xt[("v", c, qs)][:, 128 * r:128 * (r + 1)],
                        rhs=wv_t[:, c, :],
                        start=(c == 0), stop=(c == CCH - 1))
                nc.vector.tensor_copy(
                    vh[kr][:, :, 64:128],
                    pv[:, 0, 0:FS].rearrange("p (h d) -> p h d", d=DEPTH))

            # ---------- attention unit (logits + dups + exp + mask) ----
            pend_all = {0: {p: [] for p in range(PAIRS)}}

            def attn_unit(qcb, kc, pair, dups):
                qsl = slice(QC * qcb, QC * (qcb + 1))
                ksl = slice(128 * kc, 128 * (kc + 1))
                lg2 = big()
                for half in range(2):
                    psl = slice(64 * half, 64 * (half + 1))
                    nc.tensor.matmul(
                        lg2[:, half, :],
                        lhsT=khT[pair][psl, ksl],
                        rhs=qhT[pair][psl, qsl],
                        start=True, stop=True)
                # idempotent dup matmuls: PE-warmth filler in stretches
                # where the PE would otherwise idle and HAM would throttle
                for _ in range(dups):
                    for half in range(2):
                        psl = slice(64 * half, 64 * (half + 1))
                        nc.tensor.matmul(
                            lg2[:, half, 0:DUP_N],
                            lhsT=khT[pair][psl, ksl],
                            rhs=qhT[pair][psl, QC * qcb:QC * qcb + DUP_N],
                            start=True, stop=True)
                ex2 = exp_p.tile([128, 2, QC], f16, tag="ex", name="ex")
                nc.scalar.activation(ex2[:], lg2[:], Act.Exp, scale=0.125)
                exm2 = exm_p.tile([128, 2, QC], f16, tag="exm", name="exm")
                mbc = (mk[(qcb, kc)][:]
                       .rearrange("p (o q) -> p o q", o=1)
                       .to_broadcast((128, 2, QC)))
                nc.vector.tensor_mul(exm2[:], ex2[:], mbc)
                pend_all[qcb][pair].append((qcb, pair, kc, exm2))

            # ---------- lead projections (ACT idle here, bias on ACT).
            # m0 halves + the first attention unit first, so the exp chain
            # starts as soon as the m0 pieces and first mask are in.
            proj_piece(khT, wk_t, "k", 0, 0, bk_t, True, warm=WARM_K)
            proj_piece(qhT, wq_t, "q", 0, 0, bq_t, True, warm=WARM_Q)
            attn_unit(0, 0, 0, 0)
            proj_piece(khT, wk_t, "k", 0, 1, bk_t, True)
            proj_piece(qhT, wq_t, "q", 0, 1, bq_t, True)

            # ---------- rest of the input DMAs, deadline order ----------
            for c in range(CCH):
                x_dma(xk, "k", c, 1)
            wv_t = wvp.tile([128, CCH, FS], f16, tag="w", name="w")
            nc.sync.dma_start(wv_t[:], wvd[:])
            for c in range(CCH):
                x_dma(xv, "v", c, 0)
            for kc in range(4, 8):
                mk_dma(0, kc)
            for c in range(CCH):
                x_dma(xk, "k", c, 2)
            for c in range(CCH):
                x_dma(xv, "v", c, 1)
            for kc in range(8, 12):
                mk_dma(0, kc)
            for c in range(CCH):
                x_dma(xk, "k", c, 3)
            for c in range(CCH):
                x_dma(xv, "v", c, 2)
            wo_t = []
            for p in range(PAIRS):
                t = wop.tile([128, DM], f16, tag="wo", name="wo")
                nc.sync.dma_start(t[:], wod[p])
                wo_t.append(t)
            for c in range(CCH):
                x_dma(xq, "q", c, 1)
            for c in range(CCH):
                x_dma(xv, "v", c, 3)
            for kc in range(12, 16):
                mk_dma(0, kc)

            # ---------- attention ----------
            av2 = {}      # qcb -> [pair tiles]
            attnN2 = {}   # qcb -> [pair an tiles]

            def emit_av(qcb, pair, dk, exm2):
                if qcb not in av2:   # lazy: bind PSUM slots at first use
                    av2[qcb] = [big() for _ in range(PAIRS)]
                for half in range(2):
                    nc.tensor.matmul(
                        av2[qcb][pair][:, half, :],
                        lhsT=vh[dk][:, 2 * pair + half, :],
                        rhs=exm2[:, half, :],
                        start=(dk == 0), stop=(dk == NKC - 1),
                        skip_group_check=True)

            def normalize(qcb, pair, act_ok=False):
                # ScalarE is the bottleneck engine mid-kernel, so the two
                # 64-partition shifts go DVE -> (SBUF-to-SBUF DMA) instead
                # of scalar.copy; in the tail ACT is idle and helps out.
                av = av2[qcb][pair]
                au2 = aup.tile([128, QC], f32, tag="au", name="au")
                rc2 = rcp.tile([128, QC], f32, tag="rc", name="rc")
                if act_ok:
                    nc.scalar.copy(au2[0:64, :], av[64:128, 0, :])
                    nc.scalar.copy(au2[64:128, :], av[64:128, 1, :])
                else:
                    tmp = aup.tile([128, QC], f32, tag="au", name="au")
                    nc.vector.tensor_copy(tmp[64:128, :], av[64:128, 0, :])
                    nc.sync.dma_start(au2[0:64, :], tmp[64:128, :])
                    nc.vector.tensor_copy(au2[64:128, :], av[64:128, 1, :])
                nc.vector.reciprocal_approx_fast(rc2[0:64, :], av[0:64, 0, :])
                rcb = rcp.tile([128, QC], f32, tag="rc", name="rc")
                nc.vector.reciprocal_approx_fast(rcb[0:64, :], av[0:64, 1, :])
                nc.sync.dma_start(rc2[64:128, :], rcb[0:64, :])
                an2 = anp.tile([128, QC], f16, tag="an", name="an")
                nc.vector.tensor_mul(an2[:], au2[:], rc2[:])
                return an2

            def emit_wo_qm(qcb, qm, act_cast=False):
                row = slice(128 * (4 * qcb + qm), 128 * (4 * qcb + qm + 1))
                po = big()
                for dn in range(2):
                    dsl = slice(512 * dn, 512 * (dn + 1))
                    for p in range(PAIRS):
                        nc.tensor.matmul(
                            po[:, dn, :],
                            lhsT=attnN2[qcb][p][:, 128 * qm:128 * (qm + 1)],
                            rhs=wo_t[p][:, dsl],
                            start=(p == 0), stop=(p == PAIRS - 1))
                ot = osp.tile([128, 2, 512], f16, tag="os", name="os")
                if act_cast:
                    nc.scalar.copy(ot[:], po[:])
                else:
                    nc.vector.tensor_copy(ot[:], po[:])
                nc.sync.dma_start(
                    out[row, :].rearrange("p (o q) -> p o q", o=2), ot[:])

            # Hook tables: extra PE work interleaved into the kc loops.
            # ("K",qs,m) / ("Q",qs,m): projection halves; ("V",kr): V chunk;
            # ("AVC",): drain up to 2 carried AVs per pair; ("NORM",qcb):
            # both normalizes; ("WO",qcb,qm): output projection block;
            # ("XDMA",key,qs): deferred input DMA emission.
            #
            # Block-lagged AV: block q's AVs all run in block q+1 (drained
            # 2/pair at kc0-7).  That evens PE load across blocks (block 0
            # is full of K/Q/V projections) and means only ONE av2
            # generation is PSUM-live at a time: lg rotation (2 slots) +
            # av2[q-1] (2 slots), with WO/Q-proj tiles reusing av2[q-1]'s
            # slots right after NORM frees them at kc8.
            hooks = {}
            # block 0: K qs1-3 early (deadlines kc4/8/12), Q qs1 next
            # (deadline b1 kc0), V spread out (deadline: b1 kc0-7 drain)
            hooks[(0, 1)] = [("K", 1, 0)]
            hooks[(0, 2)] = [("K", 1, 1), ("V", 0)]
            hooks[(0, 3)] = [("K", 2, 0), ("V", 1)]
            hooks[(0, 4)] = [("K", 2, 1), ("V", 2)]
            hooks[(0, 5)] = [("K", 3, 0), ("V", 3)]
            hooks[(0, 6)] = [("K", 3, 1), ("V", 4)]
            hooks[(0, 7)] = [("Q", 1, 0), ("V", 5)]
            hooks[(0, 8)] = [("Q", 1, 1), ("V", 6)]
            hooks[(0, 9)] = [("V", 7), ("V", 8)]
            hooks[(0, 10)] = [("V", 9), ("V", 10), ("XDMA", "q", 2)]
            hooks[(0, 11)] = [("V", 11), ("V", 12)]
            hooks[(0, 12)] = [("V", 13)]
            hooks[(0, 13)] = [("V", 14)]
            hooks[(0, 14)] = [("V", 15)]
            # blocks 1-3 drain the previous block's AVs first (AVC, 2/pair
            # per kc), then normalize/WO; later blocks also pop some of
            # their own AVs so block 3 + tail don't inherit all the work
            for kc in range(8):
                hooks.setdefault((1, kc), []).append(("AVC", 2))
            hooks.setdefault((1, 8), []).append(("NORM", 0))
            for i, kc in enumerate((9, 10, 11, 12)):
                hooks.setdefault((1, kc), []).append(("WO", 0, i))
            hooks.setdefault((1, 2), []).append(("XDMA", "q", 3))
            hooks.setdefault((1, 13), []).append(("Q", 2, 0))
            hooks.setdefault((1, 14), []).append(("Q", 2, 1))

            for kc in range(7):
                hooks.setdefault((2, kc), []).append(("AVC", 2))
            hooks.setdefault((2, 7), []).append(("NORM", 1))
            for i, kc in enumerate((8, 10, 12, 13)):
                hooks.setdefault((2, kc), []).append(("WO", 1, i))
            hooks.setdefault((2, 14), []).append(("Q", 3, 0))
            hooks.setdefault((2, 15), []).append(("Q", 3, 1))

            for kc in range(6):
                hooks.setdefault((3, kc), []).append(("AVC", 2))
            hooks.setdefault((3, 6), []).append(("AVC", 1))
            hooks.setdefault((3, 7), []).append(("NORM", 2))
            for i, kc in enumerate((8, 10, 12, 14)):
                hooks.setdefault((3, kc), []).append(("WO", 2, i))

            carry = {p: [] for p in range(PAIRS)}   # AVs deferred across blocks

            def run_hook(h):
                if h[0] == "K":
                    proj_piece(khT, wk_t, "k", h[1], h[2], bk_t, False)
                elif h[0] == "Q":
                    proj_piece(qhT, wq_t, "q", h[1], h[2], bq_t, False)
                elif h[0] == "V":
                    v_kr(h[1])
                elif h[0] == "AVC":
                    for p in range(PAIRS):
                        for _ in range(h[1]):
                            if carry[p]:
                                emit_av(*carry[p].pop(0))
                elif h[0] == "NORM":
                    attnN2[h[1]] = [normalize(h[1], p) for p in range(PAIRS)]
                elif h[0] == "WO":
                    emit_wo_qm(h[1], h[2])
                elif h[0] == "XDMA":
                    for c in range(CCH):
                        x_dma(xq, h[1], c, h[2])

            HOOK_EST = {"K": 1750, "Q": 1750, "V": 870, "WO": 864,
                        "NORM": 0, "XDMA": 0}

            def dup_count(qcb, kc):
                if qcb == 0:
                    return 2 if kc == 0 else DUP_B0
                est = 430 + 864 * own_pops(qcb, kc)
                for h in hooks.get((qcb, kc), []):
                    est += h[1] * 864 if h[0] == "AVC" else HOOK_EST[h[0]]
                return max(0, min(8, (2104 - est) * DUPSCALE // 21500))

            # own-block AV pops: only block 3 drains itself, so the tail
            # stays short; blocks 0-2 carry everything forward
            def own_pops(qcb, kc):
                if qcb == 1:
                    return 2 if kc == 15 else 0
                if qcb == 2:
                    return 1 if kc in (9, 11, 13) else 0
                if qcb == 3:
                    if kc in (9, 11, 13, 15):
                        return 2
                    return 1 if kc == 6 else 0
                return 0

            for qcb in range(NQC):
                pend = pend_all.setdefault(
                    qcb, {p: [] for p in range(PAIRS)})
                last = qcb == NQC - 1
                for kc in range(NKC):
                    if qcb < NQC - 1 and kc >= 8:  # prefetch next block masks
                        mk_dma(qcb + 1, 2 * (kc - 8))
                        mk_dma(qcb + 1, 2 * (kc - 8) + 1)
                    pops = own_pops(qcb, kc)
                    dups = dup_count(qcb, kc)
                    for pair in range(PAIRS):
                        if (qcb, kc, pair) != (0, 0, 0):  # pre-emitted in lead
                            attn_unit(qcb, kc, pair, dups)
                        for _ in range(pops):
                            if pend[pair]:
                                emit_av(*pend[pair].pop(0))
                    for h in hooks.get((qcb, kc), []):
                        run_hook(h)
                for p in range(PAIRS):
                    carry[p] = pend[p]

            # tail: drain the rest of block 3's AVs with PE fill through
            # the final normalize, then the last WO blocks
            for p in range(PAIRS):
                while pend_all[NQC - 1][p]:
                    emit_av(*pend_all[NQC - 1][p].pop(0))
            wu2 = big()
            for i in range(TAIL):
                nc.tensor.matmul(
                    wu2[:, i % 2, :], lhsT=khT[0][:, 0:128],
                    rhs=khT[0][:, 0:QC],
                    start=True, stop=True, skip_group_check=True)
            attnN2[NQC - 1] = [normalize(NQC - 1, p, act_ok=True)
                               for p in range(PAIRS)]
            for qm in range(4):
                emit_wo_qm(NQC - 1, qm, act_cast=qm % 2 == 1)

    nc.compile()
    return nc


def _get_program():
    if "nc" not in _CACHE:
        _CACHE["nc"] = _build()
    return _CACHE["nc"]


def _in_maps(q, k, v, mask, wq, bq, wk, bk, wv, bv, wo, bo):
    q = np.asarray(q, np.float32)
    k = np.asarray(k, np.float32)
    v = np.asarray(v, np.float32)
    mask = np.asarray(mask, np.float32)
    wq = np.asarray(wq, np.float32)
    wk = np.asarray(wk, np.float32)
    wv = np.asarray(wv, np.float32)
    wo = np.asarray(wo, np.float32)
    bq = np.asarray(bq, np.float32)
    bk = np.asarray(bk, np.float32)
    bv = np.asarray(bv, np.float32)
    assert np.all(bv == 0.0), "nonzero bv not supported by this kernel"

    def wdev(w, cols):
        # [128, CCH, FS] layout: partition p, contraction chunk c holds
        # dram row 128*c + p of w[cols].T
        wT = np.ascontiguousarray(w[cols].T).astype(np.float16)
        return np.ascontiguousarray(
            wT.reshape(CCH, 128, FS).transpose(1, 0, 2))

    maps = []
    xqT = [np.ascontiguousarray(q[b].T).astype(np.float16) for b in range(B)]
    xkT = [np.ascontiguousarray(k[b].T).astype(np.float16) for b in range(B)]
    xvT = [np.ascontiguousarray(v[b].T).astype(np.float16) for b in range(B)]
    m01 = [np.ascontiguousarray((1.0 - mask[b, 0]).T).astype(np.float16)
           for b in range(B)]
    for c in range(NCORES):
        b, g = divmod(c, GROUPS)
        cols = slice(FS * g, FS * (g + 1))
        maps.append({
            "xq": xqT[b], "xk": xkT[b], "xv": xvT[b],
            "wq": wdev(wq, cols),
            "wk": wdev(wk, cols),
            "wv": wdev(wv, cols),
            "wo": np.ascontiguousarray(
                wo[:, cols].T.reshape(PAIRS, 128, DM)).astype(np.float16),
            "m01": m01[b],
            "bq": np.ascontiguousarray(bq[cols].reshape(2, 128).T),
            "bk": np.ascontiguousarray(bk[cols].reshape(2, 128).T),
        })
    return maps


def _run(maps, trace=False):
    from concourse.bass_utils import run_bass_kernel_spmd
    nc = _get_program()
    kwargs = {}
    if trace:
        kwargs = dict(trace=True, tmpdir=os.environ.get("KERNEL_TRACE_DIR"))
    return run_bass_kernel_spmd(nc, maps, list(range(NCORES)), **kwargs)


def kernel(q, k, v, mask, wq, bq, wk, bk, wv, bv, wo, bo):
    maps = _in_maps(q, k, v, mask, wq, bq, wk, bk, wv, bv, wo, bo)
    res = _run(maps)
    parts = [res.results[c]["part"].astype(np.float32) for c in range(NCORES)]
    bo = np.asarray(bo, np.float32)
    outb = [parts[GROUPS * b] + parts[GROUPS * b + 1]
            + parts[GROUPS * b + 2] + parts[GROUPS * b + 3] + bo
            for b in range(B)]
    return np.stack(outb, 0).astype(np.float32)
